# revision 18
# baseline (speedup 1.0000x reference)
"""LRU layer (reset-gated complex diagonal recurrence) on 8 trn2 NeuronCores.

Strategy:
  - The mask (reset flags) is input data: the host splits the time axis AT
    RESET POSITIONS into independent segments (h_t = Bu_t at a reset, so a
    segment starting at a reset needs no incoming state). Core chunk
    boundaries are snapped to resets, so there are no cross-core carries and
    no masks inside segments.
  - Each core gets ~T/8 rows. Its segments are sorted by length (desc) and
    laid out as columns; scan step t processes the prefix of columns whose
    segment is still alive -> dense [128, n_t] vector ops, zero wasted math.
  - Host uploads the input pre-permuted AND transposed ([F, Tpad], step-major
    ragged layout), so the device only does: fp32r matmuls (Bu = Bn @ x),
    a 4-instruction-per-step complex scan (scalar_tensor_tensor on VectorE,
    lambda as per-partition scalars, H on partitions), fp32r output matmuls
    (y = Re(C h) + D x via an extra diagonal matmul), and DMA.
  - Host inverse-permutes the outputs and assembles complex64 h.

Self-contained: hardcodes T=32768, F=H=512, 8 cores (works for other sizes).
"""

import os
import sys

import numpy as np

if "/opt/trn_rl_repo" not in sys.path:
    sys.path.insert(0, "/opt/trn_rl_repo")

TRACE = bool(int(os.environ.get("KERNEL_TRACE", "0")))
LAST_RESULT = {}

F = 512
H = 512
NCORES = 8
SEG_W = 512  # column-segment width (PSUM bank / matmul free dim)


# ----------------------------------------------------------------- host prep
def _derive_params(theta_log, nu_log, gamma_log, B_real, B_imag, C_real, C_imag, D):
    lam = np.exp(-np.exp(nu_log.astype(np.float64))
                 + 1j * np.exp(theta_log.astype(np.float64)))
    gam = np.exp(gamma_log.astype(np.float64))
    bn = (B_real.astype(np.float64) + 1j * B_imag.astype(np.float64)) * gam[:, None]
    out = {
        "lam_re": lam.real.astype(np.float32),
        "lam_im": lam.imag.astype(np.float32),
        # lhsT layouts (contraction dim on partitions):
        "bre": np.ascontiguousarray(bn.real.T.astype(np.float32)),      # [F,H]
        "bim": np.ascontiguousarray(bn.imag.T.astype(np.float32)),      # [F,H]
        "cre": np.ascontiguousarray(C_real.T.astype(np.float32)),       # [H,F]
        "cimn": np.ascontiguousarray((-C_imag).T.astype(np.float32)),   # [H,F]
    }
    dd = np.zeros((128, F), dtype=np.float32)
    for fb in range(F // 128):
        blk = D.astype(np.float32)[fb * 128:(fb + 1) * 128]
        dd[np.arange(128), fb * 128 + np.arange(128)] = blk
    out["dd"] = dd
    return out


def _schedule(mask, T):
    """Split [0,T) at resets into per-core segment lists + common padded plan."""
    m = np.asarray(mask).astype(bool)
    resets = np.flatnonzero(m)
    # core boundaries snapped to resets
    bounds = [0]
    for k in range(1, NCORES):
        tgt = k * T // NCORES
        i = np.searchsorted(resets, tgt)
        cand = []
        if i < len(resets):
            cand.append(int(resets[i]))
        if i > 0:
            cand.append(int(resets[i - 1]))
        cand = [c for c in cand if c > bounds[-1]]
        if not cand:
            cand = [min(bounds[-1] + 1, T - 1)]
        bounds.append(min(cand, key=lambda r: abs(r - tgt)))
    bounds.append(T)

    cores = []
    for k in range(NCORES):
        lo, hi = bounds[k], bounds[k + 1]
        starts = np.unique(np.concatenate(
            [[lo], resets[(resets > lo) & (resets < hi)]])).astype(np.int64)
        lens = np.diff(np.concatenate([starts, [hi]])).astype(np.int64)
        gate = lens.copy()
        if k == 0:
            # carry-seeded first segment: force it to column 0 by gating it
            # as the longest segment (pad columns beyond its real length are
            # discarded via the permutation).
            i0 = int(np.where(starts == lo)[0][0])
            gate[i0] = max(int(lens.max()), int(lens[i0])) + 1
        order = np.argsort(-gate, kind="stable")
        cores.append({"starts": starts[order], "lens": lens[order],
                      "gate": gate[order], "lo": lo, "hi": hi})

    lmax = max(int(c["gate"].max()) for c in cores)
    n_t = np.zeros((NCORES, lmax), dtype=np.int64)
    for k, c in enumerate(cores):
        for t in range(lmax):
            n_t[k, t] = int((c["gate"] > t).sum())
    N_t = n_t.max(axis=0)  # common schedule
    N_t = N_t[N_t > 0]
    N_t = N_t + (N_t % 2)  # fp32r matmul needs even free dim
    lmax = len(N_t)
    off = np.zeros(lmax + 1, dtype=np.int64)
    off[1:] = np.cumsum(N_t)
    tpad = int(off[-1])

    # per-core permutation: perm[j] = original global row, or -1 (pad)
    perms = []
    for k, c in enumerate(cores):
        perm = np.full(tpad, -1, dtype=np.int64)
        for t in range(lmax):
            alive = c["gate"] > t          # sorted desc -> prefix
            nk = int(alive.sum())
            if nk == 0:
                continue
            real = c["lens"][:nk] > t      # real row exists (carry-seg gating)
            cols = off[t] + np.arange(nk)
            rows = c["starts"][:nk] + t
            perm[cols[real]] = rows[real]
        perms.append(perm)

    jobs = []  # (t, flat0, prev_flat0 (-1 if t==0), w)
    for t in range(lmax):
        nt = int(N_t[t])
        for c0 in range(0, nt, SEG_W):
            w = min(SEG_W, nt - c0)
            prev = int(off[t - 1] + c0) if t > 0 else -1
            jobs.append((t, int(off[t] + c0), prev, w))
    return {"tpad": tpad, "jobs": jobs, "perms": perms, "lmax": lmax,
            "N_t": N_t, "off": off, "bounds": bounds}


def _pack_core_inputs(inputs, carry, mask, params, sched, k):
    T = inputs.shape[0]
    tpad = sched["tpad"]
    perm = sched["perms"][k]
    valid = perm >= 0
    xt = np.zeros((F, tpad), dtype=np.float32)
    xt[:, valid] = inputs[perm[valid]].T

    lam_t = np.zeros((128, 12), dtype=np.float32)
    for hb in range(H // 128):
        lam_t[:, hb] = params["lam_re"][hb * 128:(hb + 1) * 128]
        lam_t[:, 4 + hb] = params["lam_im"][hb * 128:(hb + 1) * 128]
        lam_t[:, 8 + hb] = -params["lam_im"][hb * 128:(hb + 1) * 128]

    cfx = np.zeros((128, 8), dtype=np.float32)
    if k == 0 and not bool(mask[0]):
        lam = params["lam_re"].astype(np.float64) + 1j * params["lam_im"]
        seed = lam * carry.reshape(-1).astype(np.float64)
        for hb in range(H // 128):
            cfx[:, hb] = seed.real[hb * 128:(hb + 1) * 128].astype(np.float32)
            cfx[:, 4 + hb] = seed.imag[hb * 128:(hb + 1) * 128].astype(np.float32)

    return {"xt": xt, "bre": params["bre"], "bim": params["bim"],
            "cre": params["cre"], "cimn": params["cimn"], "dd": params["dd"],
            "lam": lam_t, "cfx": cfx}


# ------------------------------------------------------------- device program
def _build_nc(sched):
    import concourse.bacc as bacc
    import concourse.mybir as mybir
    from concourse.tile import TileContext
    from contextlib import ExitStack

    dt32 = mybir.dt.float32
    dtr = mybir.dt.float32r
    MULT = mybir.AluOpType.mult
    ADD = mybir.AluOpType.add
    tpad = sched["tpad"]
    jobs = sched["jobs"]

    nc = bacc.Bacc()
    xt_d = nc.dram_tensor("xt", [F, tpad], dtr, kind="ExternalInput")
    bre_d = nc.dram_tensor("bre", [F, H], dtr, kind="ExternalInput")
    bim_d = nc.dram_tensor("bim", [F, H], dtr, kind="ExternalInput")
    cre_d = nc.dram_tensor("cre", [H, F], dtr, kind="ExternalInput")
    cimn_d = nc.dram_tensor("cimn", [H, F], dtr, kind="ExternalInput")
    dd_d = nc.dram_tensor("dd", [128, F], dtr, kind="ExternalInput")
    lam_d = nc.dram_tensor("lam", [128, 12], dt32, kind="ExternalInput")
    cfx_d = nc.dram_tensor("cfx", [128, 8], dt32, kind="ExternalInput")
    hre_d = nc.dram_tensor("hre", [H, tpad], dt32, kind="ExternalOutput")
    him_d = nc.dram_tensor("him", [H, tpad], dt32, kind="ExternalOutput")
    y_d = nc.dram_tensor("y", [F, tpad], dt32, kind="ExternalOutput")

    with ExitStack() as ctx:
        tc = ctx.enter_context(TileContext(nc))
        wpool = ctx.enter_context(tc.tile_pool(name="w", bufs=1))
        bigpool = ctx.enter_context(tc.tile_pool(name="big", bufs=1))
        xpool = ctx.enter_context(tc.tile_pool(name="x", bufs=3))
        uvpool = ctx.enter_context(tc.tile_pool(name="uv", bufs=2))
        ypool = ctx.enter_context(tc.tile_pool(name="y", bufs=3))
        pp = ctx.enter_context(tc.tile_pool(name="ps", bufs=4, space="PSUM"))
        ppy = ctx.enter_context(tc.tile_pool(name="psy", bufs=2, space="PSUM"))

        # weights: DMA -> fp32 stage -> ACT copy -> fp32r tile.  All matmul
        # inputs are produced by ACT/DVE so each self-loading fp32r matmul
        # needs at most ONE sync wait (hardware limit).
        bw = {}
        for name, dram in (("bre", bre_d), ("bim", bim_d)):
            for kb in range(4):
                tl = wpool.tile([128, 512], dtr, tag=f"{name}{kb}", name=f"{name}{kb}")
                nc.sync.dma_start(tl[:, :], dram[kb * 128:(kb + 1) * 128, :])
                bw[(name, kb)] = tl
        lam_t = wpool.tile([128, 12], dt32, tag="lam", name="lam_t")
        nc.sync.dma_start(lam_t[:, :], lam_d[:, :])
        cfx_t = wpool.tile([128, 8], dt32, tag="cfx", name="cfx_t")
        nc.sync.dma_start(cfx_t[:, :], cfx_d[:, :])

        # persistent state buffers [128, tpad] per (h-block, re/im)
        B = {}
        for hb in range(4):
            for ci in range(2):
                B[(hb, ci)] = bigpool.tile([128, tpad], dt32,
                                           tag=f"B{hb}{ci}", name=f"B{hb}{ci}")

        # --- phase A: Bu matmuls in full-width strips (step-agnostic) ----
        strips = [(c0, min(SEG_W, tpad - c0)) for c0 in range(0, tpad, SEG_W)]
        for (c0, w) in strips:
            xws = []
            for fb in range(4):
                xw = xpool.tile([128, SEG_W], dtr, tag=f"xw{fb}", name=f"xw{fb}")
                nc.sync.dma_start(xw[:, :w],
                                  xt_d[fb * 128:(fb + 1) * 128, c0:c0 + w])
                xws.append(xw)
            for hb in range(4):
                for ci, wname in ((0, "bre"), (1, "bim")):
                    ps = pp.tile([128, SEG_W], dt32, tag="ps", name="ps")
                    for kb in range(4):
                        nc.tensor.matmul(
                            ps[:, :w],
                            bw[(wname, kb)][:, hb * 128:(hb + 1) * 128],
                            xws[kb][:, :w],
                            start=(kb == 0), stop=(kb == 3))
                    dst = B[(hb, ci)][:, c0:c0 + w].bitcast(dtr)
                    if c0 == 0:
                        nc.vector.tensor_copy(dst, ps[:, :w])
                    else:
                        nc.scalar.copy(dst, ps[:, :w])
            if c0 == 0:
                # carry seed into column 0 (zero data on cores 1..7)
                for hb in range(4):
                    nc.vector.tensor_add(B[(hb, 0)][:, 0:1].bitcast(dtr),
                                         B[(hb, 0)][:, 0:1], cfx_t[:, hb:hb + 1])
                    nc.vector.tensor_add(B[(hb, 1)][:, 0:1].bitcast(dtr),
                                         B[(hb, 1)][:, 0:1], cfx_t[:, 4 + hb:5 + hb])

        # h columns of step 0 are final after phase A: stream them out
        off1 = int(sched["off"][1]) if sched["lmax"] > 1 else tpad
        for c0 in range(0, off1, SEG_W):
            wjob = min(SEG_W, off1 - c0)
            for hb in range(4):
                nc.sync.dma_start(hre_d[hb * 128:(hb + 1) * 128, c0:c0 + wjob],
                                  B[(hb, 0)][:, c0:c0 + wjob])
                nc.sync.dma_start(him_d[hb * 128:(hb + 1) * 128, c0:c0 + wjob],
                                  B[(hb, 1)][:, c0:c0 + wjob])

        # --- phase B: scan, per (step, segment); h streams out per step --
        for (t, flat0, prev0, w) in jobs:
            if t < 1:
                continue
            for hb in range(4):
                bre_s = B[(hb, 0)][:, flat0:flat0 + w]
                bim_s = B[(hb, 1)][:, flat0:flat0 + w]
                hre_p = B[(hb, 0)][:, prev0:prev0 + w]
                him_p = B[(hb, 1)][:, prev0:prev0 + w]
                u = uvpool.tile([128, SEG_W], dt32, tag="u", name="u")
                v = uvpool.tile([128, SEG_W], dt32, tag="v", name="v")
                l_re = lam_t[:, hb:hb + 1]
                l_im = lam_t[:, 4 + hb:5 + hb]
                l_mim = lam_t[:, 8 + hb:9 + hb]
                nc.vector.scalar_tensor_tensor(
                    u[:, :w], him_p, l_mim, bre_s, op0=MULT, op1=ADD)
                nc.vector.scalar_tensor_tensor(
                    v[:, :w], hre_p, l_im, bim_s, op0=MULT, op1=ADD)
                nc.vector.scalar_tensor_tensor(
                    bre_s.bitcast(dtr), hre_p, l_re, u[:, :w],
                    op0=MULT, op1=ADD)
                nc.vector.scalar_tensor_tensor(
                    bim_s.bitcast(dtr), him_p, l_re, v[:, :w],
                    op0=MULT, op1=ADD)
                nc.sync.dma_start(hre_d[hb * 128:(hb + 1) * 128,
                                        flat0:flat0 + w], bre_s)
                nc.sync.dma_start(him_d[hb * 128:(hb + 1) * 128,
                                        flat0:flat0 + w], bim_s)

        for name, dram in (("cre", cre_d), ("cimn", cimn_d)):
            for kb in range(4):
                tl = wpool.tile([128, 512], dtr, tag=f"{name}{kb}", name=f"{name}{kb}")
                nc.sync.dma_start(tl[:, :], dram[kb * 128:(kb + 1) * 128, :])
                bw[(name, kb)] = tl
        ddw = wpool.tile([128, F], dtr, tag="dd", name="ddw")
        nc.sync.dma_start(ddw[:, :], dd_d[:, :])

        # --- phase C: outputs in full-width strips ----------------------
        for (c0, w) in strips:
            xws = []
            for fb in range(4):
                xw = xpool.tile([128, SEG_W], dtr, tag=f"xw{fb}", name=f"xw{fb}")
                nc.sync.dma_start(xw[:, :w],
                                  xt_d[fb * 128:(fb + 1) * 128, c0:c0 + w])
                xws.append(xw)
            for fb in range(4):
                psy = ppy.tile([128, SEG_W], dt32, tag="psy", name="psy")
                nc.tensor.matmul(
                    psy[:, :w], ddw[:, fb * 128:(fb + 1) * 128], xws[fb][:, :w],
                    start=True, stop=False)
                for kb in range(4):
                    nc.tensor.matmul(
                        psy[:, :w], bw[("cre", kb)][:, fb * 128:(fb + 1) * 128],
                        B[(kb, 0)][:, c0:c0 + w].bitcast(dtr),
                        start=False, stop=False)
                    nc.tensor.matmul(
                        psy[:, :w], bw[("cimn", kb)][:, fb * 128:(fb + 1) * 128],
                        B[(kb, 1)][:, c0:c0 + w].bitcast(dtr),
                        start=False, stop=(kb == 3))
                yt = ypool.tile([128, SEG_W], dt32, tag="y", name="yt")
                nc.scalar.copy(yt[:, :w], psy[:, :w])
                nc.sync.dma_start(y_d[fb * 128:(fb + 1) * 128, c0:c0 + w],
                                  yt[:, :w])
    return nc


# ------------------------------------------------------------------ frontend
def kernel(inputs, mask, carry, theta_log, nu_log, gamma_log,
           B_real, B_imag, C_real, C_imag, D):
    inputs = np.asarray(inputs, dtype=np.float32)
    mask = np.asarray(mask)
    T = inputs.shape[0]
    params = _derive_params(np.asarray(theta_log), np.asarray(nu_log),
                            np.asarray(gamma_log), np.asarray(B_real),
                            np.asarray(B_imag), np.asarray(C_real),
                            np.asarray(C_imag), np.asarray(D))
    if int((np.asarray(mask) != 0).sum()) < 2 * NCORES:
        return _numpy_fallback(inputs, mask, np.asarray(carry), params)

    sched = _schedule(mask, T)
    in_maps = [_pack_core_inputs(inputs, np.asarray(carry), mask, params,
                                 sched, k) for k in range(NCORES)]

    if TRACE:
        _install_ntff_hook_shim()
    from concourse.bass_utils import run_bass_kernel_spmd
    nc = _build_nc(sched)
    if not nc.is_finalized():
        nc.finalize()
    res = run_bass_kernel_spmd(nc, in_maps, core_ids=list(range(NCORES)),
                               trace=TRACE)
    LAST_RESULT["exec_time_ns"] = res.exec_time_ns
    LAST_RESULT["mean_exec_time_ns"] = res.mean_exec_time_ns
    LAST_RESULT["trace"] = res.instructions_and_trace

    h = np.empty((T, H), dtype=np.complex64)
    y = np.empty((T, F), dtype=np.float32)
    for k in range(NCORES):
        perm = sched["perms"][k]
        valid = perm >= 0
        rows = perm[valid]
        r = res.results[k]
        h[rows] = (r["hre"][:, valid] + 1j * r["him"][:, valid]).T
        y[rows] = r["y"][:, valid].T
    return (h, y)


def _install_ntff_hook_shim():
    """The image's antenv lacks axon_hooks; recreate the tiny get/set registry
    and register the ctypes NTFF hook so trace=True works under axon."""
    import types
    try:
        from antenv.axon_hooks import get_axon_ntff_profile_hook  # noqa: F401
        return  # already present
    except ImportError:
        pass
    try:
        import antenv
        mod = types.ModuleType("antenv.axon_hooks")
        _h = [None]
        mod.set_axon_ntff_profile_hook = lambda hook: _h.__setitem__(0, hook)
        mod.get_axon_ntff_profile_hook = lambda: _h[0]
        sys.modules["antenv.axon_hooks"] = mod
        antenv.axon_hooks = mod
        if "/root/.axon_site" not in sys.path:
            sys.path.insert(0, "/root/.axon_site")
        from trn_agent_boot.trn_boot import _ntff_profile_via_ctypes
        mod.set_axon_ntff_profile_hook(
            _ntff_profile_via_ctypes("/opt/axon/libaxon_pjrt.so"))
        import concourse.bass_utils as bu
        bu.upload_artifacts = lambda tmpdir: f"local://{tmpdir}"  # no S3 here
    except Exception as e:  # profiling is best-effort
        print("ntff hook shim failed:", e)


def _numpy_fallback(inputs, mask, carry, params):
    """Degenerate-mask path (never hit for the real data): exact but on host."""
    T = inputs.shape[0]
    lam = params["lam_re"].astype(np.float64) + 1j * params["lam_im"]
    bn_t = params["bre"].astype(np.float64) + 1j * params["bim"]  # [F,H] = Bn.T
    bu = inputs.astype(np.float64) @ bn_t
    h = np.empty((T, H), dtype=np.complex128)
    state = carry.reshape(-1).astype(np.complex128)
    mm = np.asarray(mask) != 0
    for t in range(T):
        state = bu[t] if mm[t] else lam * state + bu[t]
        h[t] = state
    cre = params["cre"].astype(np.float64)   # [H,F] = C_re.T
    cim = -params["cimn"].astype(np.float64)
    y = h.real @ cre - h.imag @ cim
    fb = np.arange(F)
    dvec = params["dd"][fb % 128, fb]
    y = y + dvec[None, :] * inputs.astype(np.float64)
    return (h.astype(np.complex64), y.astype(np.float32))


# revision 19
# speedup vs baseline: 1.5329x; 1.5329x over previous
"""LRU layer (reset-gated complex diagonal recurrence) on 8 trn2 NeuronCores.

Strategy:
  - The mask (reset flags) is input data: the host splits the time axis AT
    RESET POSITIONS into independent segments (h_t = Bu_t at a reset, so a
    segment starting at a reset needs no incoming state). Core chunk
    boundaries are snapped to resets, so there are no cross-core carries and
    no masks inside segments.
  - Each core gets ~T/8 rows. Its segments are sorted by length (desc) and
    laid out as columns; scan step t processes the prefix of columns whose
    segment is still alive -> dense [128, n_t] vector ops, zero wasted math.
  - Host uploads the input pre-permuted AND transposed ([F, Tpad], step-major
    ragged layout), so the device only does: fp32r matmuls (Bu = Bn @ x),
    a 4-instruction-per-step complex scan (scalar_tensor_tensor on VectorE,
    lambda as per-partition scalars, H on partitions), fp32r output matmuls
    (y = Re(C h) + D x via an extra diagonal matmul), and DMA.
  - Host inverse-permutes the outputs and assembles complex64 h.

Self-contained: hardcodes T=32768, F=H=512, 8 cores (works for other sizes).
"""

import os
import sys

import numpy as np

if "/opt/trn_rl_repo" not in sys.path:
    sys.path.insert(0, "/opt/trn_rl_repo")

TRACE = bool(int(os.environ.get("KERNEL_TRACE", "0")))
LAST_RESULT = {}

F = 512
H = 512
NCORES = 8
SEG_W = 512  # column-segment width (PSUM bank / matmul free dim)


# ----------------------------------------------------------------- host prep
def _derive_params(theta_log, nu_log, gamma_log, B_real, B_imag, C_real, C_imag, D):
    lam = np.exp(-np.exp(nu_log.astype(np.float64))
                 + 1j * np.exp(theta_log.astype(np.float64)))
    gam = np.exp(gamma_log.astype(np.float64))
    bn = (B_real.astype(np.float64) + 1j * B_imag.astype(np.float64)) * gam[:, None]
    out = {
        "lam_re": lam.real.astype(np.float32),
        "lam_im": lam.imag.astype(np.float32),
        # lhsT layouts (contraction dim on partitions):
        "bre": np.ascontiguousarray(bn.real.T.astype(np.float32)),      # [F,H]
        "bim": np.ascontiguousarray(bn.imag.T.astype(np.float32)),      # [F,H]
        "cre": np.ascontiguousarray(C_real.T.astype(np.float32)),       # [H,F]
        "cimn": np.ascontiguousarray((-C_imag).T.astype(np.float32)),   # [H,F]
    }
    dd = np.zeros((128, F), dtype=np.float32)
    for fb in range(F // 128):
        blk = D.astype(np.float32)[fb * 128:(fb + 1) * 128]
        dd[np.arange(128), fb * 128 + np.arange(128)] = blk
    out["dd"] = dd
    return out


def _schedule(mask, T):
    """Split [0,T) at resets into per-core segment lists + common padded plan."""
    m = np.asarray(mask).astype(bool)
    resets = np.flatnonzero(m)
    # core boundaries snapped to resets
    bounds = [0]
    for k in range(1, NCORES):
        tgt = k * T // NCORES
        i = np.searchsorted(resets, tgt)
        cand = []
        if i < len(resets):
            cand.append(int(resets[i]))
        if i > 0:
            cand.append(int(resets[i - 1]))
        cand = [c for c in cand if c > bounds[-1]]
        if not cand:
            cand = [min(bounds[-1] + 1, T - 1)]
        bounds.append(min(cand, key=lambda r: abs(r - tgt)))
    bounds.append(T)

    cores = []
    for k in range(NCORES):
        lo, hi = bounds[k], bounds[k + 1]
        starts = np.unique(np.concatenate(
            [[lo], resets[(resets > lo) & (resets < hi)]])).astype(np.int64)
        lens = np.diff(np.concatenate([starts, [hi]])).astype(np.int64)
        gate = lens.copy()
        if k == 0:
            # carry-seeded first segment: force it to column 0 by gating it
            # as the longest segment (pad columns beyond its real length are
            # discarded via the permutation).
            i0 = int(np.where(starts == lo)[0][0])
            gate[i0] = max(int(lens.max()), int(lens[i0])) + 1
        order = np.argsort(-gate, kind="stable")
        cores.append({"starts": starts[order], "lens": lens[order],
                      "gate": gate[order], "lo": lo, "hi": hi})

    lmax = max(int(c["gate"].max()) for c in cores)
    n_t = np.zeros((NCORES, lmax), dtype=np.int64)
    for k, c in enumerate(cores):
        for t in range(lmax):
            n_t[k, t] = int((c["gate"] > t).sum())
    N_t = n_t.max(axis=0)  # common schedule
    N_t = N_t[N_t > 0]
    N_t = N_t + (N_t % 2)  # fp32r matmul needs even free dim
    lmax = len(N_t)
    off = np.zeros(lmax + 1, dtype=np.int64)
    off[1:] = np.cumsum(N_t)
    tpad = int(off[-1])

    # per-core permutation: perm[j] = original global row, or -1 (pad)
    perms = []
    for k, c in enumerate(cores):
        perm = np.full(tpad, -1, dtype=np.int64)
        for t in range(lmax):
            alive = c["gate"] > t          # sorted desc -> prefix
            nk = int(alive.sum())
            if nk == 0:
                continue
            real = c["lens"][:nk] > t      # real row exists (carry-seg gating)
            cols = off[t] + np.arange(nk)
            rows = c["starts"][:nk] + t
            perm[cols[real]] = rows[real]
        perms.append(perm)

    jobs = []  # (t, flat0, prev_flat0 (-1 if t==0), w)
    for t in range(lmax):
        nt = int(N_t[t])
        for c0 in range(0, nt, SEG_W):
            w = min(SEG_W, nt - c0)
            prev = int(off[t - 1] + c0) if t > 0 else -1
            jobs.append((t, int(off[t] + c0), prev, w))
    return {"tpad": tpad, "jobs": jobs, "perms": perms, "lmax": lmax,
            "N_t": N_t, "off": off, "bounds": bounds}


def _pack_core_inputs(inputs, carry, mask, params, sched, k):
    T = inputs.shape[0]
    tpad = sched["tpad"]
    perm = sched["perms"][k]
    valid = perm >= 0
    xt = np.zeros((F, tpad), dtype=np.float32)
    xt[:, valid] = inputs[perm[valid]].T

    lam_t = np.zeros((128, 12), dtype=np.float32)
    for hb in range(H // 128):
        lam_t[:, hb] = params["lam_re"][hb * 128:(hb + 1) * 128]
        lam_t[:, 4 + hb] = params["lam_im"][hb * 128:(hb + 1) * 128]
        lam_t[:, 8 + hb] = -params["lam_im"][hb * 128:(hb + 1) * 128]

    cfx = np.zeros((128, 8), dtype=np.float32)
    if k == 0 and not bool(mask[0]):
        lam = params["lam_re"].astype(np.float64) + 1j * params["lam_im"]
        seed = lam * carry.reshape(-1).astype(np.float64)
        for hb in range(H // 128):
            cfx[:, hb] = seed.real[hb * 128:(hb + 1) * 128].astype(np.float32)
            cfx[:, 4 + hb] = seed.imag[hb * 128:(hb + 1) * 128].astype(np.float32)

    return {"xt": xt, "bre": params["bre"], "bim": params["bim"],
            "cre": params["cre"], "cimn": params["cimn"], "dd": params["dd"],
            "lam": lam_t, "cfx": cfx}


# ------------------------------------------------------------- device program
def _build_nc(sched):
    import concourse.bacc as bacc
    import concourse.mybir as mybir
    from concourse.tile import TileContext
    from contextlib import ExitStack

    dt32 = mybir.dt.float32
    dtr = mybir.dt.float32r
    MULT = mybir.AluOpType.mult
    ADD = mybir.AluOpType.add
    tpad = sched["tpad"]
    jobs = sched["jobs"]

    nc = bacc.Bacc()
    xt_d = nc.dram_tensor("xt", [F, tpad], dtr, kind="ExternalInput")
    bre_d = nc.dram_tensor("bre", [F, H], dtr, kind="ExternalInput")
    bim_d = nc.dram_tensor("bim", [F, H], dtr, kind="ExternalInput")
    cre_d = nc.dram_tensor("cre", [H, F], dtr, kind="ExternalInput")
    cimn_d = nc.dram_tensor("cimn", [H, F], dtr, kind="ExternalInput")
    dd_d = nc.dram_tensor("dd", [128, F], dtr, kind="ExternalInput")
    lam_d = nc.dram_tensor("lam", [128, 12], dt32, kind="ExternalInput")
    cfx_d = nc.dram_tensor("cfx", [128, 8], dt32, kind="ExternalInput")
    hre_d = nc.dram_tensor("hre", [H, tpad], dt32, kind="ExternalOutput")
    him_d = nc.dram_tensor("him", [H, tpad], dt32, kind="ExternalOutput")
    y_d = nc.dram_tensor("y", [F, tpad], dt32, kind="ExternalOutput")

    with ExitStack() as ctx:
        tc = ctx.enter_context(TileContext(nc))
        wpool = ctx.enter_context(tc.tile_pool(name="w", bufs=1))
        bigpool = ctx.enter_context(tc.tile_pool(name="big", bufs=1))
        xpool = ctx.enter_context(tc.tile_pool(name="x", bufs=3))
        uvpool = ctx.enter_context(tc.tile_pool(name="uv", bufs=2))
        ypool = ctx.enter_context(tc.tile_pool(name="y", bufs=3))
        pp = ctx.enter_context(tc.tile_pool(name="ps", bufs=4, space="PSUM"))
        ppy = ctx.enter_context(tc.tile_pool(name="psy", bufs=2, space="PSUM"))

        # weights: DMA -> fp32 stage -> ACT copy -> fp32r tile.  All matmul
        # inputs are produced by ACT/DVE so each self-loading fp32r matmul
        # needs at most ONE sync wait (hardware limit).
        bw = {}
        for name, dram in (("bre", bre_d), ("bim", bim_d)):
            for kb in range(4):
                tl = wpool.tile([128, 512], dtr, tag=f"{name}{kb}", name=f"{name}{kb}")
                nc.sync.dma_start(tl[:, :], dram[kb * 128:(kb + 1) * 128, :])
                bw[(name, kb)] = tl
        lam_t = wpool.tile([128, 12], dt32, tag="lam", name="lam_t")
        nc.sync.dma_start(lam_t[:, :], lam_d[:, :])
        cfx_t = wpool.tile([128, 8], dt32, tag="cfx", name="cfx_t")
        nc.sync.dma_start(cfx_t[:, :], cfx_d[:, :])

        # persistent state buffers [128, tpad] per (h-block, re/im)
        B = {}
        for hb in range(4):
            for ci in range(2):
                B[(hb, ci)] = bigpool.tile([128, tpad], dt32,
                                           tag=f"B{hb}{ci}", name=f"B{hb}{ci}")

        # --- phase A: Bu matmuls in full-width strips (step-agnostic) ----
        strips = [(c0, min(SEG_W, tpad - c0)) for c0 in range(0, tpad, SEG_W)]
        for (c0, w) in strips:
            xws = []
            for fb in range(4):
                xw = xpool.tile([128, SEG_W], dtr, tag=f"xw{fb}", name=f"xw{fb}")
                nc.sync.dma_start(xw[:, :w],
                                  xt_d[fb * 128:(fb + 1) * 128, c0:c0 + w])
                xws.append(xw)
            for hb in range(4):
                for ci, wname in ((0, "bre"), (1, "bim")):
                    ps = pp.tile([128, SEG_W], dt32, tag="ps", name="ps")
                    for kb in range(4):
                        nc.tensor.matmul(
                            ps[:, :w],
                            bw[(wname, kb)][:, hb * 128:(hb + 1) * 128],
                            xws[kb][:, :w],
                            start=(kb == 0), stop=(kb == 3))
                    dst = B[(hb, ci)][:, c0:c0 + w].bitcast(dtr)
                    if c0 == 0:
                        nc.vector.tensor_copy(dst, ps[:, :w])
                    else:
                        nc.scalar.copy(dst, ps[:, :w])
            if c0 == 0:
                # carry seed into column 0 (zero data on cores 1..7)
                for hb in range(4):
                    nc.vector.tensor_add(B[(hb, 0)][:, 0:1].bitcast(dtr),
                                         B[(hb, 0)][:, 0:1], cfx_t[:, hb:hb + 1])
                    nc.vector.tensor_add(B[(hb, 1)][:, 0:1].bitcast(dtr),
                                         B[(hb, 1)][:, 0:1], cfx_t[:, 4 + hb:5 + hb])

        # --- phase B: scan, per (step, segment) -------------------------
        for (t, flat0, prev0, w) in jobs:
            if t < 1:
                continue
            for hb in range(4):
                bre_s = B[(hb, 0)][:, flat0:flat0 + w]
                bim_s = B[(hb, 1)][:, flat0:flat0 + w]
                hre_p = B[(hb, 0)][:, prev0:prev0 + w]
                him_p = B[(hb, 1)][:, prev0:prev0 + w]
                u = uvpool.tile([128, SEG_W], dt32, tag="u", name="u")
                v = uvpool.tile([128, SEG_W], dt32, tag="v", name="v")
                l_re = lam_t[:, hb:hb + 1]
                l_im = lam_t[:, 4 + hb:5 + hb]
                l_mim = lam_t[:, 8 + hb:9 + hb]
                nc.vector.scalar_tensor_tensor(
                    u[:, :w], him_p, l_mim, bre_s, op0=MULT, op1=ADD)
                nc.vector.scalar_tensor_tensor(
                    v[:, :w], hre_p, l_im, bim_s, op0=MULT, op1=ADD)
                nc.vector.scalar_tensor_tensor(
                    bre_s.bitcast(dtr), hre_p, l_re, u[:, :w],
                    op0=MULT, op1=ADD)
                nc.vector.scalar_tensor_tensor(
                    bim_s.bitcast(dtr), him_p, l_re, v[:, :w],
                    op0=MULT, op1=ADD)

        for name, dram in (("cre", cre_d), ("cimn", cimn_d)):
            for kb in range(4):
                tl = wpool.tile([128, 512], dtr, tag=f"{name}{kb}", name=f"{name}{kb}")
                nc.sync.dma_start(tl[:, :], dram[kb * 128:(kb + 1) * 128, :])
                bw[(name, kb)] = tl
        ddw = wpool.tile([128, F], dtr, tag="dd", name="ddw")
        nc.sync.dma_start(ddw[:, :], dd_d[:, :])

        # --- phase C: outputs in full-width strips ----------------------
        for (c0, w) in strips:
            xws = []
            for fb in range(4):
                xw = xpool.tile([128, SEG_W], dtr, tag=f"xw{fb}", name=f"xw{fb}")
                nc.sync.dma_start(xw[:, :w],
                                  xt_d[fb * 128:(fb + 1) * 128, c0:c0 + w])
                xws.append(xw)
            for fb in range(4):
                psy = ppy.tile([128, SEG_W], dt32, tag="psy", name="psy")
                nc.tensor.matmul(
                    psy[:, :w], ddw[:, fb * 128:(fb + 1) * 128], xws[fb][:, :w],
                    start=True, stop=False)
                for kb in range(4):
                    nc.tensor.matmul(
                        psy[:, :w], bw[("cre", kb)][:, fb * 128:(fb + 1) * 128],
                        B[(kb, 0)][:, c0:c0 + w].bitcast(dtr),
                        start=False, stop=False)
                    nc.tensor.matmul(
                        psy[:, :w], bw[("cimn", kb)][:, fb * 128:(fb + 1) * 128],
                        B[(kb, 1)][:, c0:c0 + w].bitcast(dtr),
                        start=False, stop=(kb == 3))
                yt = ypool.tile([128, SEG_W], dt32, tag="y", name="yt")
                nc.scalar.copy(yt[:, :w], psy[:, :w])
                nc.sync.dma_start(y_d[fb * 128:(fb + 1) * 128, c0:c0 + w],
                                  yt[:, :w])
            for hb in range(4):
                nc.sync.dma_start(hre_d[hb * 128:(hb + 1) * 128, c0:c0 + w],
                                  B[(hb, 0)][:, c0:c0 + w])
                nc.sync.dma_start(him_d[hb * 128:(hb + 1) * 128, c0:c0 + w],
                                  B[(hb, 1)][:, c0:c0 + w])
    return nc


# ------------------------------------------------------------------ frontend
def kernel(inputs, mask, carry, theta_log, nu_log, gamma_log,
           B_real, B_imag, C_real, C_imag, D):
    inputs = np.asarray(inputs, dtype=np.float32)
    mask = np.asarray(mask)
    T = inputs.shape[0]
    params = _derive_params(np.asarray(theta_log), np.asarray(nu_log),
                            np.asarray(gamma_log), np.asarray(B_real),
                            np.asarray(B_imag), np.asarray(C_real),
                            np.asarray(C_imag), np.asarray(D))
    if int((np.asarray(mask) != 0).sum()) < 2 * NCORES:
        return _numpy_fallback(inputs, mask, np.asarray(carry), params)

    sched = _schedule(mask, T)
    in_maps = [_pack_core_inputs(inputs, np.asarray(carry), mask, params,
                                 sched, k) for k in range(NCORES)]

    if TRACE:
        _install_ntff_hook_shim()
    from concourse.bass_utils import run_bass_kernel_spmd
    nc = _build_nc(sched)
    if not nc.is_finalized():
        nc.finalize()
    res = run_bass_kernel_spmd(nc, in_maps, core_ids=list(range(NCORES)),
                               trace=TRACE)
    LAST_RESULT["exec_time_ns"] = res.exec_time_ns
    LAST_RESULT["mean_exec_time_ns"] = res.mean_exec_time_ns
    LAST_RESULT["trace"] = res.instructions_and_trace

    h = np.empty((T, H), dtype=np.complex64)
    y = np.empty((T, F), dtype=np.float32)
    for k in range(NCORES):
        perm = sched["perms"][k]
        valid = perm >= 0
        rows = perm[valid]
        r = res.results[k]
        h[rows] = (r["hre"][:, valid] + 1j * r["him"][:, valid]).T
        y[rows] = r["y"][:, valid].T
    return (h, y)


def _install_ntff_hook_shim():
    """The image's antenv lacks axon_hooks; recreate the tiny get/set registry
    and register the ctypes NTFF hook so trace=True works under axon."""
    import types
    try:
        from antenv.axon_hooks import get_axon_ntff_profile_hook  # noqa: F401
        return  # already present
    except ImportError:
        pass
    try:
        import antenv
        mod = types.ModuleType("antenv.axon_hooks")
        _h = [None]
        mod.set_axon_ntff_profile_hook = lambda hook: _h.__setitem__(0, hook)
        mod.get_axon_ntff_profile_hook = lambda: _h[0]
        sys.modules["antenv.axon_hooks"] = mod
        antenv.axon_hooks = mod
        if "/root/.axon_site" not in sys.path:
            sys.path.insert(0, "/root/.axon_site")
        from trn_agent_boot.trn_boot import _ntff_profile_via_ctypes
        mod.set_axon_ntff_profile_hook(
            _ntff_profile_via_ctypes("/opt/axon/libaxon_pjrt.so"))
        import concourse.bass_utils as bu
        bu.upload_artifacts = lambda tmpdir: f"local://{tmpdir}"  # no S3 here
    except Exception as e:  # profiling is best-effort
        print("ntff hook shim failed:", e)


def _numpy_fallback(inputs, mask, carry, params):
    """Degenerate-mask path (never hit for the real data): exact but on host."""
    T = inputs.shape[0]
    lam = params["lam_re"].astype(np.float64) + 1j * params["lam_im"]
    bn_t = params["bre"].astype(np.float64) + 1j * params["bim"]  # [F,H] = Bn.T
    bu = inputs.astype(np.float64) @ bn_t
    h = np.empty((T, H), dtype=np.complex128)
    state = carry.reshape(-1).astype(np.complex128)
    mm = np.asarray(mask) != 0
    for t in range(T):
        state = bu[t] if mm[t] else lam * state + bu[t]
        h[t] = state
    cre = params["cre"].astype(np.float64)   # [H,F] = C_re.T
    cim = -params["cimn"].astype(np.float64)
    y = h.real @ cre - h.imag @ cim
    fb = np.arange(F)
    dvec = params["dd"][fb % 128, fb]
    y = y + dvec[None, :] * inputs.astype(np.float64)
    return (h.astype(np.complex64), y.astype(np.float32))


# revision 20
# speedup vs baseline: 1.5347x; 1.0012x over previous
"""LRU layer (reset-gated complex diagonal recurrence) on 8 trn2 NeuronCores.

Strategy:
  - The mask (reset flags) is input data: the host splits the time axis AT
    RESET POSITIONS into independent segments (h_t = Bu_t at a reset, so a
    segment starting at a reset needs no incoming state). Core chunk
    boundaries are snapped to resets, so there are no cross-core carries and
    no masks inside segments.
  - Each core gets ~T/8 rows. Its segments are sorted by length (desc) and
    laid out as columns; scan step t processes the prefix of columns whose
    segment is still alive -> dense [128, n_t] vector ops, zero wasted math.
  - Host uploads the input pre-permuted AND transposed ([F, Tpad], step-major
    ragged layout), so the device only does: fp32r matmuls (Bu = Bn @ x),
    a 4-instruction-per-step complex scan (scalar_tensor_tensor on VectorE,
    lambda as per-partition scalars, H on partitions), fp32r output matmuls
    (y = Re(C h) + D x via an extra diagonal matmul), and DMA.
  - Host inverse-permutes the outputs and assembles complex64 h.

Self-contained: hardcodes T=32768, F=H=512, 8 cores (works for other sizes).
"""

import os
import sys

import numpy as np

if "/opt/trn_rl_repo" not in sys.path:
    sys.path.insert(0, "/opt/trn_rl_repo")

TRACE = bool(int(os.environ.get("KERNEL_TRACE", "0")))
LAST_RESULT = {}

F = 512
H = 512
NCORES = 8
SEG_W = 512  # column-segment width (PSUM bank / matmul free dim)


# ----------------------------------------------------------------- host prep
def _derive_params(theta_log, nu_log, gamma_log, B_real, B_imag, C_real, C_imag, D):
    lam = np.exp(-np.exp(nu_log.astype(np.float64))
                 + 1j * np.exp(theta_log.astype(np.float64)))
    gam = np.exp(gamma_log.astype(np.float64))
    bn = (B_real.astype(np.float64) + 1j * B_imag.astype(np.float64)) * gam[:, None]
    out = {
        "lam_re": lam.real.astype(np.float32),
        "lam_im": lam.imag.astype(np.float32),
        # lhsT layouts (contraction dim on partitions):
        "bre": np.ascontiguousarray(bn.real.T.astype(np.float32)),      # [F,H]
        "bim": np.ascontiguousarray(bn.imag.T.astype(np.float32)),      # [F,H]
        "cre": np.ascontiguousarray(C_real.T.astype(np.float32)),       # [H,F]
        "cimn": np.ascontiguousarray((-C_imag).T.astype(np.float32)),   # [H,F]
    }
    dd = np.zeros((128, F), dtype=np.float32)
    for fb in range(F // 128):
        blk = D.astype(np.float32)[fb * 128:(fb + 1) * 128]
        dd[np.arange(128), fb * 128 + np.arange(128)] = blk
    out["dd"] = dd
    return out


def _schedule(mask, T):
    """Split [0,T) at resets into per-core segment lists + common padded plan."""
    m = np.asarray(mask).astype(bool)
    resets = np.flatnonzero(m)
    # core boundaries snapped to resets
    bounds = [0]
    for k in range(1, NCORES):
        tgt = k * T // NCORES
        i = np.searchsorted(resets, tgt)
        cand = []
        if i < len(resets):
            cand.append(int(resets[i]))
        if i > 0:
            cand.append(int(resets[i - 1]))
        cand = [c for c in cand if c > bounds[-1]]
        if not cand:
            cand = [min(bounds[-1] + 1, T - 1)]
        bounds.append(min(cand, key=lambda r: abs(r - tgt)))
    bounds.append(T)

    cores = []
    for k in range(NCORES):
        lo, hi = bounds[k], bounds[k + 1]
        starts = np.unique(np.concatenate(
            [[lo], resets[(resets > lo) & (resets < hi)]])).astype(np.int64)
        lens = np.diff(np.concatenate([starts, [hi]])).astype(np.int64)
        gate = lens.copy()
        if k == 0:
            # carry-seeded first segment: force it to column 0 by gating it
            # as the longest segment (pad columns beyond its real length are
            # discarded via the permutation).
            i0 = int(np.where(starts == lo)[0][0])
            gate[i0] = max(int(lens.max()), int(lens[i0])) + 1
        order = np.argsort(-gate, kind="stable")
        cores.append({"starts": starts[order], "lens": lens[order],
                      "gate": gate[order], "lo": lo, "hi": hi})

    lmax = max(int(c["gate"].max()) for c in cores)
    n_t = np.zeros((NCORES, lmax), dtype=np.int64)
    for k, c in enumerate(cores):
        for t in range(lmax):
            n_t[k, t] = int((c["gate"] > t).sum())
    N_t = n_t.max(axis=0)  # common schedule
    N_t = N_t[N_t > 0]
    N_t = N_t + (N_t % 2)  # fp32r matmul needs even free dim
    lmax = len(N_t)
    off = np.zeros(lmax + 1, dtype=np.int64)
    off[1:] = np.cumsum(N_t)
    tpad = int(off[-1])

    # per-core permutation: perm[j] = original global row, or -1 (pad)
    perms = []
    for k, c in enumerate(cores):
        perm = np.full(tpad, -1, dtype=np.int64)
        for t in range(lmax):
            alive = c["gate"] > t          # sorted desc -> prefix
            nk = int(alive.sum())
            if nk == 0:
                continue
            real = c["lens"][:nk] > t      # real row exists (carry-seg gating)
            cols = off[t] + np.arange(nk)
            rows = c["starts"][:nk] + t
            perm[cols[real]] = rows[real]
        perms.append(perm)

    jobs = []  # (t, flat0, prev_flat0 (-1 if t==0), w)
    for t in range(lmax):
        nt = int(N_t[t])
        for c0 in range(0, nt, SEG_W):
            w = min(SEG_W, nt - c0)
            prev = int(off[t - 1] + c0) if t > 0 else -1
            jobs.append((t, int(off[t] + c0), prev, w))
    return {"tpad": tpad, "jobs": jobs, "perms": perms, "lmax": lmax,
            "N_t": N_t, "off": off, "bounds": bounds}


def _pack_core_inputs(inputs, carry, mask, params, sched, k):
    T = inputs.shape[0]
    tpad = sched["tpad"]
    perm = sched["perms"][k]
    valid = perm >= 0
    xt = np.zeros((F, tpad), dtype=np.float32)
    xt[:, valid] = inputs[perm[valid]].T

    lam_t = np.zeros((128, 12), dtype=np.float32)
    for hb in range(H // 128):
        lam_t[:, hb] = params["lam_re"][hb * 128:(hb + 1) * 128]
        lam_t[:, 4 + hb] = params["lam_im"][hb * 128:(hb + 1) * 128]
        lam_t[:, 8 + hb] = -params["lam_im"][hb * 128:(hb + 1) * 128]

    cfx = np.zeros((128, 8), dtype=np.float32)
    if k == 0 and not bool(mask[0]):
        lam = params["lam_re"].astype(np.float64) + 1j * params["lam_im"]
        seed = lam * carry.reshape(-1).astype(np.float64)
        for hb in range(H // 128):
            cfx[:, hb] = seed.real[hb * 128:(hb + 1) * 128].astype(np.float32)
            cfx[:, 4 + hb] = seed.imag[hb * 128:(hb + 1) * 128].astype(np.float32)

    return {"xt": xt, "bre": params["bre"], "bim": params["bim"],
            "cre": params["cre"], "cimn": params["cimn"], "dd": params["dd"],
            "lam": lam_t, "cfx": cfx}


# ------------------------------------------------------------- device program
def _build_nc(sched):
    import concourse.bacc as bacc
    import concourse.mybir as mybir
    from concourse.tile import TileContext
    from contextlib import ExitStack

    dt32 = mybir.dt.float32
    dtr = mybir.dt.float32r
    MULT = mybir.AluOpType.mult
    ADD = mybir.AluOpType.add
    tpad = sched["tpad"]
    jobs = sched["jobs"]

    nc = bacc.Bacc()
    xt_d = nc.dram_tensor("xt", [F, tpad], dtr, kind="ExternalInput")
    bre_d = nc.dram_tensor("bre", [F, H], dtr, kind="ExternalInput")
    bim_d = nc.dram_tensor("bim", [F, H], dtr, kind="ExternalInput")
    cre_d = nc.dram_tensor("cre", [H, F], dtr, kind="ExternalInput")
    cimn_d = nc.dram_tensor("cimn", [H, F], dtr, kind="ExternalInput")
    dd_d = nc.dram_tensor("dd", [128, F], dtr, kind="ExternalInput")
    lam_d = nc.dram_tensor("lam", [128, 12], dt32, kind="ExternalInput")
    cfx_d = nc.dram_tensor("cfx", [128, 8], dt32, kind="ExternalInput")
    hre_d = nc.dram_tensor("hre", [H, tpad], dt32, kind="ExternalOutput")
    him_d = nc.dram_tensor("him", [H, tpad], dt32, kind="ExternalOutput")
    y_d = nc.dram_tensor("y", [F, tpad], dt32, kind="ExternalOutput")

    with ExitStack() as ctx:
        tc = ctx.enter_context(TileContext(nc))
        wpool = ctx.enter_context(tc.tile_pool(name="w", bufs=1))
        bigpool = ctx.enter_context(tc.tile_pool(name="big", bufs=1))
        xpool = ctx.enter_context(tc.tile_pool(name="x", bufs=3))
        uvpool = ctx.enter_context(tc.tile_pool(name="uv", bufs=2))
        ypool = ctx.enter_context(tc.tile_pool(name="y", bufs=3))
        pp = ctx.enter_context(tc.tile_pool(name="ps", bufs=4, space="PSUM"))
        ppy = ctx.enter_context(tc.tile_pool(name="psy", bufs=2, space="PSUM"))

        # weights: DMA -> fp32 stage -> ACT copy -> fp32r tile.  All matmul
        # inputs are produced by ACT/DVE so each self-loading fp32r matmul
        # needs at most ONE sync wait (hardware limit).
        bw = {}
        for name, dram in (("bre", bre_d), ("bim", bim_d)):
            for kb in range(4):
                tl = wpool.tile([128, 512], dtr, tag=f"{name}{kb}", name=f"{name}{kb}")
                nc.scalar.dma_start(tl[:, :], dram[kb * 128:(kb + 1) * 128, :])
                bw[(name, kb)] = tl
        lam_t = wpool.tile([128, 12], dt32, tag="lam", name="lam_t")
        nc.sync.dma_start(lam_t[:, :], lam_d[:, :])
        cfx_t = wpool.tile([128, 8], dt32, tag="cfx", name="cfx_t")
        nc.sync.dma_start(cfx_t[:, :], cfx_d[:, :])

        # persistent state buffers [128, tpad] per (h-block, re/im)
        B = {}
        for hb in range(4):
            for ci in range(2):
                B[(hb, ci)] = bigpool.tile([128, tpad], dt32,
                                           tag=f"B{hb}{ci}", name=f"B{hb}{ci}")

        # --- phase A: Bu matmuls in full-width strips (step-agnostic) ----
        strips = [(c0, min(SEG_W, tpad - c0)) for c0 in range(0, tpad, SEG_W)]
        for (c0, w) in strips:
            xws = []
            for fb in range(4):
                xw = xpool.tile([128, SEG_W], dtr, tag=f"xw{fb}", name=f"xw{fb}")
                nc.sync.dma_start(xw[:, :w],
                                  xt_d[fb * 128:(fb + 1) * 128, c0:c0 + w])
                xws.append(xw)
            for hb in range(4):
                for ci, wname in ((0, "bre"), (1, "bim")):
                    ps = pp.tile([128, SEG_W], dt32, tag="ps", name="ps")
                    for kb in range(4):
                        nc.tensor.matmul(
                            ps[:, :w],
                            bw[(wname, kb)][:, hb * 128:(hb + 1) * 128],
                            xws[kb][:, :w],
                            start=(kb == 0), stop=(kb == 3))
                    dst = B[(hb, ci)][:, c0:c0 + w].bitcast(dtr)
                    if c0 == 0:
                        nc.vector.tensor_copy(dst, ps[:, :w])
                    else:
                        nc.scalar.copy(dst, ps[:, :w])
            if c0 == 0:
                # carry seed into column 0 (zero data on cores 1..7)
                for hb in range(4):
                    nc.vector.tensor_add(B[(hb, 0)][:, 0:1].bitcast(dtr),
                                         B[(hb, 0)][:, 0:1], cfx_t[:, hb:hb + 1])
                    nc.vector.tensor_add(B[(hb, 1)][:, 0:1].bitcast(dtr),
                                         B[(hb, 1)][:, 0:1], cfx_t[:, 4 + hb:5 + hb])

        # --- phase B: scan, per (step, segment) -------------------------
        for (t, flat0, prev0, w) in jobs:
            if t < 1:
                continue
            for hb in range(4):
                bre_s = B[(hb, 0)][:, flat0:flat0 + w]
                bim_s = B[(hb, 1)][:, flat0:flat0 + w]
                hre_p = B[(hb, 0)][:, prev0:prev0 + w]
                him_p = B[(hb, 1)][:, prev0:prev0 + w]
                u = uvpool.tile([128, SEG_W], dt32, tag="u", name="u")
                v = uvpool.tile([128, SEG_W], dt32, tag="v", name="v")
                l_re = lam_t[:, hb:hb + 1]
                l_im = lam_t[:, 4 + hb:5 + hb]
                l_mim = lam_t[:, 8 + hb:9 + hb]
                nc.vector.scalar_tensor_tensor(
                    u[:, :w], him_p, l_mim, bre_s, op0=MULT, op1=ADD)
                nc.vector.scalar_tensor_tensor(
                    v[:, :w], hre_p, l_im, bim_s, op0=MULT, op1=ADD)
                nc.vector.scalar_tensor_tensor(
                    bre_s.bitcast(dtr), hre_p, l_re, u[:, :w],
                    op0=MULT, op1=ADD)
                nc.vector.scalar_tensor_tensor(
                    bim_s.bitcast(dtr), him_p, l_re, v[:, :w],
                    op0=MULT, op1=ADD)

        for name, dram in (("cre", cre_d), ("cimn", cimn_d)):
            for kb in range(4):
                tl = wpool.tile([128, 512], dtr, tag=f"{name}{kb}", name=f"{name}{kb}")
                nc.scalar.dma_start(tl[:, :], dram[kb * 128:(kb + 1) * 128, :])
                bw[(name, kb)] = tl
        ddw = wpool.tile([128, F], dtr, tag="dd", name="ddw")
        nc.scalar.dma_start(ddw[:, :], dd_d[:, :])

        # --- phase C: outputs in full-width strips ----------------------
        for (c0, w) in strips:
            xws = []
            for fb in range(4):
                xw = xpool.tile([128, SEG_W], dtr, tag=f"xw{fb}", name=f"xw{fb}")
                nc.scalar.dma_start(xw[:, :w],
                                    xt_d[fb * 128:(fb + 1) * 128, c0:c0 + w])
                xws.append(xw)
            for fb in range(4):
                psy = ppy.tile([128, SEG_W], dt32, tag="psy", name="psy")
                nc.tensor.matmul(
                    psy[:, :w], ddw[:, fb * 128:(fb + 1) * 128], xws[fb][:, :w],
                    start=True, stop=False)
                for kb in range(4):
                    nc.tensor.matmul(
                        psy[:, :w], bw[("cre", kb)][:, fb * 128:(fb + 1) * 128],
                        B[(kb, 0)][:, c0:c0 + w].bitcast(dtr),
                        start=False, stop=False)
                    nc.tensor.matmul(
                        psy[:, :w], bw[("cimn", kb)][:, fb * 128:(fb + 1) * 128],
                        B[(kb, 1)][:, c0:c0 + w].bitcast(dtr),
                        start=False, stop=(kb == 3))
                yt = ypool.tile([128, SEG_W], dt32, tag="y", name="yt")
                nc.scalar.copy(yt[:, :w], psy[:, :w])
                nc.sync.dma_start(y_d[fb * 128:(fb + 1) * 128, c0:c0 + w],
                                  yt[:, :w])
            for hb in range(4):
                nc.sync.dma_start(hre_d[hb * 128:(hb + 1) * 128, c0:c0 + w],
                                  B[(hb, 0)][:, c0:c0 + w])
                nc.sync.dma_start(him_d[hb * 128:(hb + 1) * 128, c0:c0 + w],
                                  B[(hb, 1)][:, c0:c0 + w])
    return nc


# ------------------------------------------------------------------ frontend
def kernel(inputs, mask, carry, theta_log, nu_log, gamma_log,
           B_real, B_imag, C_real, C_imag, D):
    inputs = np.asarray(inputs, dtype=np.float32)
    mask = np.asarray(mask)
    T = inputs.shape[0]
    params = _derive_params(np.asarray(theta_log), np.asarray(nu_log),
                            np.asarray(gamma_log), np.asarray(B_real),
                            np.asarray(B_imag), np.asarray(C_real),
                            np.asarray(C_imag), np.asarray(D))
    if int((np.asarray(mask) != 0).sum()) < 2 * NCORES:
        return _numpy_fallback(inputs, mask, np.asarray(carry), params)

    sched = _schedule(mask, T)
    in_maps = [_pack_core_inputs(inputs, np.asarray(carry), mask, params,
                                 sched, k) for k in range(NCORES)]

    if TRACE:
        _install_ntff_hook_shim()
    from concourse.bass_utils import run_bass_kernel_spmd
    nc = _build_nc(sched)
    if not nc.is_finalized():
        nc.finalize()
    res = run_bass_kernel_spmd(nc, in_maps, core_ids=list(range(NCORES)),
                               trace=TRACE)
    LAST_RESULT["exec_time_ns"] = res.exec_time_ns
    LAST_RESULT["mean_exec_time_ns"] = res.mean_exec_time_ns
    LAST_RESULT["trace"] = res.instructions_and_trace

    h = np.empty((T, H), dtype=np.complex64)
    y = np.empty((T, F), dtype=np.float32)
    for k in range(NCORES):
        perm = sched["perms"][k]
        valid = perm >= 0
        rows = perm[valid]
        r = res.results[k]
        h[rows] = (r["hre"][:, valid] + 1j * r["him"][:, valid]).T
        y[rows] = r["y"][:, valid].T
    return (h, y)


def _install_ntff_hook_shim():
    """The image's antenv lacks axon_hooks; recreate the tiny get/set registry
    and register the ctypes NTFF hook so trace=True works under axon."""
    import types
    try:
        from antenv.axon_hooks import get_axon_ntff_profile_hook  # noqa: F401
        return  # already present
    except ImportError:
        pass
    try:
        import antenv
        mod = types.ModuleType("antenv.axon_hooks")
        _h = [None]
        mod.set_axon_ntff_profile_hook = lambda hook: _h.__setitem__(0, hook)
        mod.get_axon_ntff_profile_hook = lambda: _h[0]
        sys.modules["antenv.axon_hooks"] = mod
        antenv.axon_hooks = mod
        if "/root/.axon_site" not in sys.path:
            sys.path.insert(0, "/root/.axon_site")
        from trn_agent_boot.trn_boot import _ntff_profile_via_ctypes
        mod.set_axon_ntff_profile_hook(
            _ntff_profile_via_ctypes("/opt/axon/libaxon_pjrt.so"))
        import concourse.bass_utils as bu
        bu.upload_artifacts = lambda tmpdir: f"local://{tmpdir}"  # no S3 here
    except Exception as e:  # profiling is best-effort
        print("ntff hook shim failed:", e)


def _numpy_fallback(inputs, mask, carry, params):
    """Degenerate-mask path (never hit for the real data): exact but on host."""
    T = inputs.shape[0]
    lam = params["lam_re"].astype(np.float64) + 1j * params["lam_im"]
    bn_t = params["bre"].astype(np.float64) + 1j * params["bim"]  # [F,H] = Bn.T
    bu = inputs.astype(np.float64) @ bn_t
    h = np.empty((T, H), dtype=np.complex128)
    state = carry.reshape(-1).astype(np.complex128)
    mm = np.asarray(mask) != 0
    for t in range(T):
        state = bu[t] if mm[t] else lam * state + bu[t]
        h[t] = state
    cre = params["cre"].astype(np.float64)   # [H,F] = C_re.T
    cim = -params["cimn"].astype(np.float64)
    y = h.real @ cre - h.imag @ cim
    fb = np.arange(F)
    dvec = params["dd"][fb % 128, fb]
    y = y + dvec[None, :] * inputs.astype(np.float64)
    return (h.astype(np.complex64), y.astype(np.float32))
